# revision 15
# baseline (speedup 1.0000x reference)
"""Trainium2 Bass kernel: batched multi-head attention with per-frame
conditioning K/V token (nn_Attention dense_transformer problem).

Strategy: data-parallel over the 16 (b*n) frames -> 2 frames per NeuronCore,
no collectives. Per core, a fused kernel:
  QKV projection (q,k feature-major; v token-major) -> per-head attention with
  sim computed transposed (keys on partitions) so softmax denominators come
  from a ones-column in the PV matmul -> output projection.
QKV chunk emission is interleaved with attention blocks per head-pair so the
scalar engine (exp) starts early and stays busy.

Layout notes:
 - All matmul operands bf16 (f32 PSUM accumulation). Host pre-transposes x to
   feature-major and pre-splits d into 128-row chunks, so no on-device
   transposes are needed anywhere.
 - Keys padded: col T = conditioning token, cols T+1..T+127 zero dummies. The
   dummy keys' v rows AND ones-column entries are zero, so they contribute
   nothing to the attention output or the softmax denominator.
 - v stored interleaved [8 heads x 72 cols] (64 v + ones-col at 64 + 7 pad) so
   each head's PV stationary operand is a contiguous (128, 65) slice at a
   16B-aligned offset (HW requirement for matmul weights); PV output row 64
   accumulates the softmax denominator for free.
"""

import numpy as np
import ml_dtypes

import concourse.bacc as bacc
import concourse.tile as tile
from concourse import mybir
from concourse.bass_utils import run_bass_kernel_spmd

BF16 = mybir.dt.bfloat16
F32 = mybir.dt.float32

HEADS = 8
DH = 64
D = 512
HID = 512
SCALE = DH ** -0.5
N_CORES = 8
NDC = D // 128  # 4 contraction chunks of 128


def build_attention_nc(T=1024, loop_n=1):
    S = T + 128             # keys T, cond at col T, 127 zero dummies
    JC = S // 128           # key chunks (9 for T=1024)
    NI = min(512, T)        # i-tile width (matmul moving free dim)
    NIH = T // NI           # i-tiles per frame
    NTC = T // 128          # token chunks (for v / out-proj)

    # split key chunks into groups of <=3 (exp batching; 3 PSUM banks/group)
    groups = [list(range(g, min(g + 3, JC))) for g in range(0, JC, 3)]

    nc = bacc.Bacc("TRN2", target_bir_lowering=False)
    x_d = nc.declare_dram_parameter("xT", [128, NDC, 2, T], BF16, isOutput=False)
    w_d = nc.declare_dram_parameter("Wqkv", [128, NDC, 3 * HID], BF16, isOutput=False)
    wk_d = nc.declare_dram_parameter("Wk", [128, NDC, HID], BF16, isOutput=False)
    wv_d = nc.declare_dram_parameter("Wv", [128, NDC, HID], BF16, isOutput=False)
    wo_d = nc.declare_dram_parameter("Wout", [128, NDC, D], BF16, isOutput=False)
    lab_d = nc.declare_dram_parameter("labT", [128, NDC, 2, 8], BF16, isOutput=False)
    f_d = nc.declare_dram_parameter("F", [33, 128], BF16, isOutput=False)
    out_d = nc.declare_dram_parameter("out", [2, T, D], F32, isOutput=True)

    EXP = mybir.ActivationFunctionType.Exp

    with tile.TileContext(nc) as tc:
        with (
            tc.tile_pool(name="persist", bufs=1) as pp,
            tc.tile_pool(name="work", bufs=4) as wp,
            tc.tile_pool(name="psum", bufs=2, space="PSUM") as psp,
        ):
            def emit_body():
                # ---- persistent SBUF tiles ----
                xT = pp.tile([128, NDC, 2, T], BF16, tag="xT")
                wq = pp.tile([128, NDC, 3 * HID], BF16, tag="wq")
                wk = pp.tile([128, NDC, HID], BF16, tag="wk")
                wv = pp.tile([128, NDC, HID], BF16, tag="wv")
                wo = pp.tile([128, NDC, D], BF16, tag="wo")
                lab = pp.tile([128, NDC, 2, 8], BF16, tag="lab")
                qT = pp.tile([128, NDC, 2, T], BF16, tag="qT")
                kT = pp.tile([128, NDC, 2, S], BF16, tag="kT")
                # 72*2B = 144B: 16B-aligned per-head stride (HW weight req)
                vv = pp.tile([128, 2, JC, HEADS, 72], BF16, tag="vv")
                attn = pp.tile([128, NDC, 2, T], BF16, tag="attn")
                fmat = pp.tile([33, 128], BF16, tag="fmat")
                # 1/denom rows: 0 (h1) and 32 (h2); rows 1-31 stay 1.0
                rg = pp.tile([33, NI], BF16, tag="rg")

                nc.sync.dma_start(wq[:], w_d[:])
                for dc in range(NDC):
                    nc.sync.dma_start(xT[:, dc], x_d[:, dc])
                nc.sync.dma_start(wk[:], wk_d[:])
                nc.sync.dma_start(wv[:], wv_d[:])
                nc.sync.dma_start(wo[:], wo_d[:])
                nc.sync.dma_start(lab[:], lab_d[:])
                nc.sync.dma_start(fmat[:], f_d[:])

                # constants / padding init (rg rows 1-31 finite; F rows 0 there)
                nc.vector.memset(rg[:], 1.0)
                nc.vector.memset(kT[:, :, :, T + 1:S], 0.0)  # dummy keys
                nc.vector.memset(vv[:], 0.0)
                nc.vector.memset(vv[:, :, 0:JC - 1, :, DH:DH + 1], 1.0)  # ones (real keys)
                nc.vector.memset(vv[0:1, :, JC - 1, :, DH:DH + 1], 1.0)  # cond token only

                def emit_v(f):
                    for tc_i in range(NTC):
                        ps = psp.tile([128, HID], F32, tag="sim")
                        for dc in range(NDC):
                            nc.tensor.matmul(
                                ps[:],
                                xT[:, dc, f, tc_i * 128:(tc_i + 1) * 128],
                                wq[:, dc, 2 * HID:3 * HID],
                                start=(dc == 0), stop=(dc == NDC - 1),
                            )
                        nc.vector.tensor_copy(vv[:, f, tc_i, :, 0:DH], ps[:])

                def emit_ek():
                    for cc in range(NDC):
                        ps = psp.tile([128, 2], F32, tag="pv")
                        for dc in range(NDC):
                            nc.tensor.matmul(
                                ps[:],
                                wk[:, dc, cc * 128:(cc + 1) * 128],
                                lab[:, dc, :, 0:1],
                                start=(dc == 0), stop=(dc == NDC - 1),
                            )
                        for f in range(2):
                            nc.vector.tensor_copy(kT[:, cc, f, T:T + 1], ps[:, f:f + 1])

                def emit_ev(f):
                    ps = psp.tile([1, HID], F32, tag="pv")
                    for dc in range(NDC):
                        nc.tensor.matmul(
                            ps[:],
                            lab[:, dc, f, 0:1],
                            wv[:, dc, :],
                            start=(dc == 0), stop=(dc == NDC - 1),
                        )
                    nc.vector.tensor_copy(vv[0:1, f, JC - 1, :, 0:DH], ps[:])

                def emit_qk(f, cc):
                    for ih in range(NIH):
                        ps = psp.tile([128, NI], F32, tag="sim")
                        for dc in range(NDC):
                            nc.tensor.matmul(
                                ps[:],
                                wq[:, dc, cc * 128:(cc + 1) * 128],
                                xT[:, dc, f, ih * NI:(ih + 1) * NI],
                                start=(dc == 0), stop=(dc == NDC - 1),
                            )
                        if cc < 4:
                            nc.vector.tensor_copy(qT[:, cc, f, ih * NI:(ih + 1) * NI], ps[:])
                        else:
                            nc.vector.tensor_copy(kT[:, cc - 4, f, ih * NI:(ih + 1) * NI], ps[:])

                def emit_block(f, a, ih):
                    isl = slice(ih * NI, (ih + 1) * NI)
                    pvA = psp.tile([65, NI], F32, tag="pv")
                    pvB = psp.tile([65, NI], F32, tag="pv")
                    for jcs in groups:
                        g = len(jcs)
                        sA = psp.tile([128, 3, NI], F32, tag="sim")
                        sB = psp.tile([128, 3, NI], F32, tag="sim")
                        for idx, jc in enumerate(jcs):
                            jsl = slice(jc * 128, (jc + 1) * 128)
                            nc.tensor.matmul(
                                sA[:, idx, :], kT[0:64, a, f, jsl], qT[0:64, a, f, isl],
                                start=True, stop=True, tile_position=(0, 0),
                            )
                            nc.tensor.matmul(
                                sB[:, idx, :], kT[64:128, a, f, jsl], qT[64:128, a, f, isl],
                                start=True, stop=True, tile_position=(64, 0),
                            )
                        pA = wp.tile([128, 3, NI], BF16, tag="P")
                        pB = wp.tile([128, 3, NI], BF16, tag="P")
                        nc.scalar.activation(pA[:, 0:g, :], sA[:, 0:g, :], EXP, scale=SCALE)
                        nc.scalar.activation(pB[:, 0:g, :], sB[:, 0:g, :], EXP, scale=SCALE)
                        for idx, jc in enumerate(jcs):
                            nc.tensor.matmul(
                                pvA[:], vv[:, f, jc, 2 * a, 0:65], pA[:, idx, :],
                                start=(jc == 0), stop=(jc == JC - 1),
                            )
                            nc.tensor.matmul(
                                pvB[:], vv[:, f, jc, 2 * a + 1, 0:65], pB[:, idx, :],
                                start=(jc == 0), stop=(jc == JC - 1),
                            )
                    # reciprocal softmax denominators (dummy keys contribute 0)
                    with nc.allow_low_precision("softmax denom reciprocal in bf16"):
                        nc.vector.reciprocal(rg[0:1, :], pvA[64:65, :])
                        nc.vector.reciprocal(rg[32:33, :], pvB[64:65, :])
                    # broadcast 1/denom across partitions: h1 -> 0:64, h2 -> 64:128
                    bc = psp.tile([128, NI], F32, tag="sim")
                    nc.tensor.matmul(bc[:], fmat[:], rg[:], start=True, stop=True)
                    rbc = wp.tile([128, NI], BF16, tag="rbc")
                    nc.vector.tensor_copy(rbc[:], bc[:])
                    # normalize + store to attn (feature-major, pair-stacked)
                    nc.vector.tensor_mul(attn[0:64, a, f, isl], pvA[0:64, :], rbc[0:64, :])
                    nc.vector.tensor_mul(attn[64:128, a, f, isl], pvB[0:64, :], rbc[64:128, :])

                def emit_proj(f):
                    for ic in range(NTC):
                        ps = psp.tile([128, D], F32, tag="pv")
                        for a in range(NDC):
                            nc.tensor.matmul(
                                ps[:],
                                attn[:, a, f, ic * 128:(ic + 1) * 128],
                                wo[:, a, :],
                                start=(a == 0), stop=(a == NDC - 1),
                            )
                        ot = wp.tile([128, D], F32, tag="oout")
                        nc.vector.tensor_copy(ot[:], ps[:])
                        nc.sync.dma_start(out_d[f, ic * 128:(ic + 1) * 128, :], ot[:])

                # interleaved emission: v/ek/ev first; each pair's qk chunks are
                # emitted during the previous pair's blocks so PE always has
                # fill work while ACT runs exp; frame-0 projection is folded
                # into frame-1's attention.
                emit_ek()
                for f in range(2):
                    emit_v(f)
                    emit_ev(f)
                    emit_qk(f, 0)
                    emit_qk(f, 4)
                    for a in range(4):
                        emit_block(f, a, 0)
                        if a < 3:
                            emit_qk(f, a + 1)
                            emit_qk(f, a + 5)
                        for ih in range(1, NIH):
                            emit_block(f, a, ih)
                        if f == 1 and a == 0:
                            emit_proj(0)
                emit_proj(1)

            if loop_n > 1:
                with tc.For_i(0, loop_n, 1):
                    emit_body()
            else:
                emit_body()

    nc.finalize()
    return nc


_NC_CACHE = {}


def _get_nc(T):
    if T not in _NC_CACHE:
        _NC_CACHE[T] = build_attention_nc(T)
    return _NC_CACHE[T]


def make_in_maps(x, label_emb_mm, Wqkv, Wk, Wv, Wout):
    """Host-side sharding + layout prep (transpose to feature-major, bf16)."""
    bf = ml_dtypes.bfloat16
    BN, T, d = x.shape
    assert (BN, d) == (16, D)
    # x[fr, t, dc*128+p] -> xB[fr, p, dc, t]
    xB = np.ascontiguousarray(
        np.asarray(x).reshape(16, T, NDC, 128).transpose(0, 3, 2, 1)
    ).astype(bf)
    wq = np.ascontiguousarray(np.asarray(Wqkv).reshape(NDC, 128, 3 * HID).transpose(1, 0, 2)).astype(bf)
    wkh = np.ascontiguousarray(np.asarray(Wk).reshape(NDC, 128, HID).transpose(1, 0, 2)).astype(bf)
    wvh = np.ascontiguousarray(np.asarray(Wv).reshape(NDC, 128, HID).transpose(1, 0, 2)).astype(bf)
    woh = np.ascontiguousarray(np.asarray(Wout).reshape(NDC, 128, D).transpose(1, 0, 2)).astype(bf)
    labB = np.asarray(label_emb_mm).reshape(16, NDC, 128)  # [fr, dc, p]
    F = np.zeros((33, 128), dtype=bf)
    F[0, 0:64] = 1.0
    F[32, 64:128] = 1.0
    in_maps = []
    for c in range(N_CORES):
        xTc = np.ascontiguousarray(xB[2 * c:2 * c + 2].transpose(1, 2, 0, 3))  # (128,4,2,T)
        labc2 = np.ascontiguousarray(labB[2 * c:2 * c + 2].transpose(2, 1, 0)).astype(bf)  # (128,4,2)
        labc = np.zeros((128, NDC, 2, 8), dtype=bf)  # padded so f-stride is 16B
        labc[:, :, :, 0] = labc2
        in_maps.append({
            "xT": xTc, "Wqkv": wq, "Wk": wkh, "Wv": wvh, "Wout": woh, "labT": labc,
            "F": F,
        })
    return in_maps


def kernel(x, label_emb_mm, Wqkv, Wk, Wv, Wout, b):
    x = np.asarray(x)
    T = x.shape[1]
    nc = _get_nc(T)
    in_maps = make_in_maps(x, label_emb_mm, Wqkv, Wk, Wv, Wout)
    res = run_bass_kernel_spmd(nc, in_maps, core_ids=list(range(N_CORES)))
    out = np.concatenate([res.results[c]["out"] for c in range(N_CORES)], axis=0)
    return np.ascontiguousarray(out.reshape(16, T, D)).astype(np.float32)


# revision 17
# speedup vs baseline: 1.1367x; 1.1367x over previous
"""Trainium2 Bass kernel: batched multi-head attention with per-frame
conditioning K/V token (nn_Attention dense_transformer problem).

Strategy: data-parallel over the 16 (b*n) frames -> 2 frames per NeuronCore,
no collectives. Per core, a fused kernel:
  QKV projection (q,k feature-major; v token-major) -> per-head attention with
  sim computed transposed (keys on partitions) so softmax denominators come
  from a ones-column in the PV matmul -> output projection.
QKV chunk emission is interleaved with attention blocks per head-pair so the
scalar engine (exp) starts early and stays busy.

Layout notes:
 - All matmul operands bf16 (f32 PSUM accumulation). Host pre-transposes x to
   feature-major and pre-splits d into 128-row chunks, so no on-device
   transposes are needed anywhere.
 - Keys padded: col T = conditioning token, cols T+1..T+127 zero dummies. The
   dummy keys' v rows AND ones-column entries are zero, so they contribute
   nothing to the attention output or the softmax denominator.
 - v stored interleaved [8 heads x 72 cols] (64 v + ones-col at 64 + 7 pad) so
   each head's PV stationary operand is a contiguous (128, 65) slice at a
   16B-aligned offset (HW requirement for matmul weights); PV output row 64
   accumulates the softmax denominator for free.
"""

import numpy as np
import ml_dtypes

import concourse.bacc as bacc
import concourse.tile as tile
from concourse import mybir
from concourse.bass_utils import run_bass_kernel_spmd

BF16 = mybir.dt.bfloat16
F32 = mybir.dt.float32

HEADS = 8
DH = 64
D = 512
HID = 512
SCALE = DH ** -0.5
N_CORES = 8
NDC = D // 128  # 4 contraction chunks of 128


def build_attention_nc(T=1024, loop_n=1):
    S = T + 128             # keys T, cond at col T, 127 zero dummies
    JC = S // 128           # key chunks (9 for T=1024)
    NI = min(512, T)        # i-tile width (matmul moving free dim)
    NIH = T // NI           # i-tiles per frame
    NTC = T // 128          # token chunks (for v / out-proj)

    nc = bacc.Bacc("TRN2", target_bir_lowering=False)
    x_d = nc.declare_dram_parameter("xT", [128, NDC, 2, T], BF16, isOutput=False)
    w_d = nc.declare_dram_parameter("Wqkv", [128, NDC, 3 * HID], BF16, isOutput=False)
    wk_d = nc.declare_dram_parameter("Wk", [128, NDC, HID], BF16, isOutput=False)
    wv_d = nc.declare_dram_parameter("Wv", [128, NDC, HID], BF16, isOutput=False)
    wo_d = nc.declare_dram_parameter("Wout", [128, NDC, D], BF16, isOutput=False)
    lab_d = nc.declare_dram_parameter("labT", [128, NDC, 2, 8], BF16, isOutput=False)
    f_d = nc.declare_dram_parameter("F", [33, 128], BF16, isOutput=False)
    out_d = nc.declare_dram_parameter("out", [2, T, D], F32, isOutput=True)

    EXP = mybir.ActivationFunctionType.Exp

    with tile.TileContext(nc) as tc:
        with (
            tc.tile_pool(name="persist", bufs=1) as pp,
            tc.tile_pool(name="work", bufs=8) as wp,
            tc.tile_pool(name="psum", bufs=4, space="PSUM") as psp,
        ):
            def emit_body():
                # ---- persistent SBUF tiles ----
                xT = pp.tile([128, NDC, 2, T], BF16, tag="xT")
                wq = pp.tile([128, NDC, 3 * HID], BF16, tag="wq")
                wk = pp.tile([128, NDC, HID], BF16, tag="wk")
                wv = pp.tile([128, NDC, HID], BF16, tag="wv")
                wo = pp.tile([128, NDC, D], BF16, tag="wo")
                lab = pp.tile([128, NDC, 2, 8], BF16, tag="lab")
                qT = pp.tile([128, NDC, 2, T], BF16, tag="qT")
                kT = pp.tile([128, NDC, 2, S], BF16, tag="kT")
                # 72*2B = 144B: 16B-aligned per-head stride (HW weight req)
                vv = pp.tile([128, 2, JC, HEADS, 72], BF16, tag="vv")
                attn = pp.tile([128, NDC, 2, T], BF16, tag="attn")
                fmat = pp.tile([33, 128], BF16, tag="fmat")
                # 1/denom rows: 0 (h1) and 32 (h2); rows 1-31 stay 1.0
                rg = pp.tile([33, NI], BF16, tag="rg")

                nc.sync.dma_start(wq[:], w_d[:])
                for dc in range(NDC):
                    nc.sync.dma_start(xT[:, dc], x_d[:, dc])
                nc.sync.dma_start(wk[:], wk_d[:])
                nc.sync.dma_start(wv[:], wv_d[:])
                nc.sync.dma_start(wo[:], wo_d[:])
                nc.sync.dma_start(lab[:], lab_d[:])
                nc.sync.dma_start(fmat[:], f_d[:])

                # constants / padding init (rg rows 1-31 finite; F rows 0 there)
                nc.vector.memset(rg[:], 1.0)
                nc.vector.memset(kT[:, :, :, T + 1:S], 0.0)  # dummy keys
                nc.vector.memset(vv[:], 0.0)
                nc.vector.memset(vv[:, :, 0:JC - 1, :, DH:DH + 1], 1.0)  # ones (real keys)
                nc.vector.memset(vv[0:1, :, JC - 1, :, DH:DH + 1], 1.0)  # cond token only

                def emit_v(f):
                    for tc_i in range(NTC):
                        ps = psp.tile([128, HID], F32, tag="sim")
                        for dc in range(NDC):
                            nc.tensor.matmul(
                                ps[:],
                                xT[:, dc, f, tc_i * 128:(tc_i + 1) * 128],
                                wq[:, dc, 2 * HID:3 * HID],
                                start=(dc == 0), stop=(dc == NDC - 1),
                            )
                        nc.vector.tensor_copy(vv[:, f, tc_i, :, 0:DH], ps[:])

                def emit_ek():
                    for cc in range(NDC):
                        ps = psp.tile([128, 2], F32, tag="pv")
                        for dc in range(NDC):
                            nc.tensor.matmul(
                                ps[:],
                                wk[:, dc, cc * 128:(cc + 1) * 128],
                                lab[:, dc, :, 0:1],
                                start=(dc == 0), stop=(dc == NDC - 1),
                            )
                        for f in range(2):
                            nc.vector.tensor_copy(kT[:, cc, f, T:T + 1], ps[:, f:f + 1])

                def emit_ev(f):
                    ps = psp.tile([1, HID], F32, tag="pv")
                    for dc in range(NDC):
                        nc.tensor.matmul(
                            ps[:],
                            lab[:, dc, f, 0:1],
                            wv[:, dc, :],
                            start=(dc == 0), stop=(dc == NDC - 1),
                        )
                    nc.vector.tensor_copy(vv[0:1, f, JC - 1, :, 0:DH], ps[:])

                def emit_qk(f, cc):
                    for ih in range(NIH):
                        ps = psp.tile([128, NI], F32, tag="sim")
                        for dc in range(NDC):
                            nc.tensor.matmul(
                                ps[:],
                                wq[:, dc, cc * 128:(cc + 1) * 128],
                                xT[:, dc, f, ih * NI:(ih + 1) * NI],
                                start=(dc == 0), stop=(dc == NDC - 1),
                            )
                        if cc < 4:
                            nc.vector.tensor_copy(qT[:, cc, f, ih * NI:(ih + 1) * NI], ps[:])
                        else:
                            nc.vector.tensor_copy(kT[:, cc - 4, f, ih * NI:(ih + 1) * NI], ps[:])

                def emit_block(f, a, ih):
                    isl = slice(ih * NI, (ih + 1) * NI)
                    pvA = psp.tile([65, NI], F32, tag="pv")
                    pvB = psp.tile([65, NI], F32, tag="pv")
                    for jc in range(JC):
                        jsl = slice(jc * 128, (jc + 1) * 128)
                        sA = psp.tile([128, NI], F32, tag="sim")
                        sB = psp.tile([128, NI], F32, tag="sim")
                        nc.tensor.matmul(
                            sA[:], kT[0:64, a, f, jsl], qT[0:64, a, f, isl],
                            start=True, stop=True, tile_position=(0, 0),
                        )
                        nc.tensor.matmul(
                            sB[:], kT[64:128, a, f, jsl], qT[64:128, a, f, isl],
                            start=True, stop=True, tile_position=(64, 0),
                        )
                        # single-bank exps: ~1.5ns/elem, no bank-crossing penalty
                        pA = wp.tile([128, NI], BF16, tag="P")
                        pB = wp.tile([128, NI], BF16, tag="P")
                        nc.scalar.activation(pA[:], sA[:], EXP, scale=SCALE)
                        nc.scalar.activation(pB[:], sB[:], EXP, scale=SCALE)
                        nc.tensor.matmul(
                            pvA[:], vv[:, f, jc, 2 * a, 0:65], pA[:],
                            start=(jc == 0), stop=(jc == JC - 1),
                        )
                        nc.tensor.matmul(
                            pvB[:], vv[:, f, jc, 2 * a + 1, 0:65], pB[:],
                            start=(jc == 0), stop=(jc == JC - 1),
                        )
                    # reciprocal softmax denominators (dummy keys contribute 0)
                    with nc.allow_low_precision("softmax denom reciprocal in bf16"):
                        nc.vector.reciprocal(rg[0:1, :], pvA[64:65, :])
                        nc.vector.reciprocal(rg[32:33, :], pvB[64:65, :])
                    # broadcast 1/denom across partitions: h1 -> 0:64, h2 -> 64:128
                    bc = psp.tile([128, NI], F32, tag="sim")
                    nc.tensor.matmul(bc[:], fmat[:], rg[:], start=True, stop=True)
                    rbc = wp.tile([128, NI], BF16, tag="rbc")
                    nc.vector.tensor_copy(rbc[:], bc[:])
                    # normalize + store to attn (feature-major, pair-stacked)
                    nc.vector.tensor_mul(attn[0:64, a, f, isl], pvA[0:64, :], rbc[0:64, :])
                    nc.vector.tensor_mul(attn[64:128, a, f, isl], pvB[0:64, :], rbc[64:128, :])

                def emit_proj(f):
                    for ic in range(NTC):
                        ps = psp.tile([128, D], F32, tag="pv")
                        for a in range(NDC):
                            nc.tensor.matmul(
                                ps[:],
                                attn[:, a, f, ic * 128:(ic + 1) * 128],
                                wo[:, a, :],
                                start=(a == 0), stop=(a == NDC - 1),
                            )
                        ot = wp.tile([128, D], F32, tag="oout")
                        nc.vector.tensor_copy(ot[:], ps[:])
                        nc.sync.dma_start(out_d[f, ic * 128:(ic + 1) * 128, :], ot[:])

                # interleaved emission: v/ek/ev first; each pair's qk chunks are
                # emitted during the previous pair's blocks so PE always has
                # fill work while ACT runs exp; frame-0 projection is folded
                # into frame-1's attention.
                emit_ek()
                for f in range(2):
                    emit_v(f)
                    emit_ev(f)
                    emit_qk(f, 0)
                    emit_qk(f, 4)
                    for a in range(4):
                        emit_block(f, a, 0)
                        if a < 3:
                            emit_qk(f, a + 1)
                            emit_qk(f, a + 5)
                        for ih in range(1, NIH):
                            emit_block(f, a, ih)
                        if f == 1 and a == 0:
                            emit_proj(0)
                emit_proj(1)

            if loop_n > 1:
                with tc.For_i(0, loop_n, 1):
                    emit_body()
            else:
                emit_body()

    nc.finalize()
    return nc


_NC_CACHE = {}


def _get_nc(T):
    if T not in _NC_CACHE:
        _NC_CACHE[T] = build_attention_nc(T)
    return _NC_CACHE[T]


def make_in_maps(x, label_emb_mm, Wqkv, Wk, Wv, Wout):
    """Host-side sharding + layout prep (transpose to feature-major, bf16)."""
    bf = ml_dtypes.bfloat16
    BN, T, d = x.shape
    assert (BN, d) == (16, D)
    # x[fr, t, dc*128+p] -> xB[fr, p, dc, t]
    xB = np.ascontiguousarray(
        np.asarray(x).reshape(16, T, NDC, 128).transpose(0, 3, 2, 1)
    ).astype(bf)
    wq = np.ascontiguousarray(np.asarray(Wqkv).reshape(NDC, 128, 3 * HID).transpose(1, 0, 2)).astype(bf)
    wkh = np.ascontiguousarray(np.asarray(Wk).reshape(NDC, 128, HID).transpose(1, 0, 2)).astype(bf)
    wvh = np.ascontiguousarray(np.asarray(Wv).reshape(NDC, 128, HID).transpose(1, 0, 2)).astype(bf)
    woh = np.ascontiguousarray(np.asarray(Wout).reshape(NDC, 128, D).transpose(1, 0, 2)).astype(bf)
    labB = np.asarray(label_emb_mm).reshape(16, NDC, 128)  # [fr, dc, p]
    F = np.zeros((33, 128), dtype=bf)
    F[0, 0:64] = 1.0
    F[32, 64:128] = 1.0
    in_maps = []
    for c in range(N_CORES):
        xTc = np.ascontiguousarray(xB[2 * c:2 * c + 2].transpose(1, 2, 0, 3))  # (128,4,2,T)
        labc2 = np.ascontiguousarray(labB[2 * c:2 * c + 2].transpose(2, 1, 0)).astype(bf)  # (128,4,2)
        labc = np.zeros((128, NDC, 2, 8), dtype=bf)  # padded so f-stride is 16B
        labc[:, :, :, 0] = labc2
        in_maps.append({
            "xT": xTc, "Wqkv": wq, "Wk": wkh, "Wv": wvh, "Wout": woh, "labT": labc,
            "F": F,
        })
    return in_maps


def kernel(x, label_emb_mm, Wqkv, Wk, Wv, Wout, b):
    x = np.asarray(x)
    T = x.shape[1]
    nc = _get_nc(T)
    in_maps = make_in_maps(x, label_emb_mm, Wqkv, Wk, Wv, Wout)
    res = run_bass_kernel_spmd(nc, in_maps, core_ids=list(range(N_CORES)))
    out = np.concatenate([res.results[c]["out"] for c in range(N_CORES)], axis=0)
    return np.ascontiguousarray(out.reshape(16, T, D)).astype(np.float32)
